# revision 48
# baseline (speedup 1.0000x reference)
"""Trainium2 Bass kernel for a single-layer bigram language model
(embed + 16-head causal attention + vocab lm_head).

Sharding: 8 cores = 4 batches x 2 vocab halves. Core c handles batch c//2
and vocab columns [(c%2)*16000, (c%2+1)*16000). Attention is replicated
across the 2 cores of a batch pair; the lm_head (dominant cost) is fully
sharded. No collectives.

The embedding lookup + positional add (a pure host-side gather over the
input tables) is done on the host; the device receives xT = (tok_emb[idx]
+ pos_emb).T pre-tiled, already bf16.

Everything on-chip is bf16 (PSUM accumulation stays fp32): same PE speed
as f32r but halves DMA/SBUF and doubles DVE throughput. Score tiles are
causally trimmed (leading fully-masked columns skipped); only a single
128x128 triangular mask remains, applied on the diagonal window.

Layout: features stay transposed ([feature, token]) so every matmul
contraction has K on partitions:
  qT/kT     = Wq_pair.T @ xT  (head pairs stacked on partitions 0:64 / 64:128)
  scoresT   = kT_h.T-slices @ qT_h  -> [s, t] tiles; exp on ACT; tri mask on DVE
  pv        = [v_h | 1].T @ e       -> rows 0:64 = unnormalized outT, row 64 = denom
  outT      = pv * (1/denom broadcast via K=1 PE matmul)
  logits    = outT.T @ lm_W + lm_b  (t on partitions, vocab on free dim)
"""

import sys

if "/opt/trn_rl_repo" not in sys.path:
    sys.path.insert(0, "/opt/trn_rl_repo")

import numpy as np

VOCAB = 32000
E = 1024
T = 1024
H = 16
HS = 64
B = 4
VSH = VOCAB // 2  # per-core vocab shard
NE = E // 128  # 8 e-tiles
NT = T // 128  # 8 t-tiles
NVC = 500  # vocab tile width
NVT = VSH // NVC  # 32 vocab tiles

_cache = {}


def _build_nc(tune=None):
    tune = dict(tune or {})

    def tn(k, d):
        return tune.get(k, d)

    import concourse.bass as bass
    import concourse.bacc as bacc
    import concourse.tile as tile
    from concourse import mybir
    from contextlib import ExitStack

    f32 = mybir.dt.float32
    f32r = mybir.dt.float32r
    bf16 = mybir.dt.bfloat16

    nc = bacc.Bacc("TRN2", target_bir_lowering=False, debug=False)

    xt_d = nc.dram_tensor("xt", [NT, 128, E], bf16, kind="ExternalInput").ap()
    wq_d = nc.dram_tensor("wq", [8, 128, NE, 128], bf16, kind="ExternalInput").ap()
    wk_d = nc.dram_tensor("wk", [8, 128, NE, 128], bf16, kind="ExternalInput").ap()
    wv_d = nc.dram_tensor("wv", [NE, 128, E], bf16, kind="ExternalInput").ap()
    tri_d = nc.dram_tensor("tri", [128, 128], bf16, kind="ExternalInput").ap()
    id64_d = nc.dram_tensor("id64", [128, 64], bf16, kind="ExternalInput").ap()
    lmw_d = nc.dram_tensor("lmw", [E, VSH], bf16, kind="ExternalInput").ap()
    lmb_d = nc.dram_tensor("lmb", [1, VSH], f32, kind="ExternalInput").ap()
    out_d = nc.dram_tensor("logits", [T, VSH], bf16, kind="ExternalOutput").ap()

    EXP = mybir.ActivationFunctionType.Exp

    with tile.TileContext(nc) as tc, ExitStack() as ctx:
        const = ctx.enter_context(tc.tile_pool(name="const", bufs=1))
        persist = ctx.enter_context(tc.tile_pool(name="persist", bufs=1))
        lwp = ctx.enter_context(tc.tile_pool(name="lwp", bufs=tn("lwp", 3)))
        bp = ctx.enter_context(tc.tile_pool(name="bp", bufs=tn("bp", 3)))

        xT = persist.tile([128, NT, NE, 128], bf16)
        outT = persist.tile([128, NE, T], bf16)
        vaug = persist.tile([128, NT, H, HS + 1], bf16)
        tri = const.tile([128, 128], bf16)
        id64 = const.tile([128, 64], bf16)
        ones64 = const.tile([128, 64], bf16)
        ogp = ctx.enter_context(tc.tile_pool(name="ogp", bufs=tn("ogp", 6)))
        wqk = ctx.enter_context(tc.tile_pool(name="wqk", bufs=tn("wqk", 4)))
        wsb_staged = {}

        def stage_w(pr):
            for wd, wtag in ((wq_d, "wq"), (wk_d, "wk")):
                w_sb = wqk.tile([128, NE, 128], bf16, tag=wtag)
                nc.sync.dma_start(
                    w_sb[:].rearrange("p k n -> p (k n)"),
                    wd[pr].rearrange("p k n -> p (k n)"),
                )
                wsb_staged[(pr, wtag)] = w_sb

        lw_staged = {}
        bias_staged = {}

        def stage_lw(v):
            lw = lwp.tile([128, NE, NVC], bf16, tag="lw")
            nc.sync.dma_start(
                lw[:],
                bass.AP(
                    tensor=lmw_d.tensor,
                    offset=v * NVC,
                    ap=[[VSH, 128], [VSH * 128, NE], [1, NVC]],
                ),
            )
            lw_staged[v] = lw
            bias = bp.tile([128, NVC], f32, tag="bias")
            nc.sync.dma_start(
                bias[:],
                bass.AP(tensor=lmb_d.tensor, offset=v * NVC, ap=[[0, 128], [1, NVC]]),
            )
            bias_staged[v] = bias

        qkp = ctx.enter_context(tc.tile_pool(name="qkp", bufs=tn("qkp", 2)))
        pqk = ctx.enter_context(tc.tile_pool(name="pqk", bufs=tn("pqk", 2), space="PSUM"))

        def emit_qk(pr):
            qT = qkp.tile([128, T], bf16, tag="qT")
            kT = qkp.tile([128, T], bf16, tag="kT")
            for wi, (wtag, dst) in enumerate((("wq", qT), ("wk", kT))):
                w_sb = wsb_staged.pop((pr, wtag))
                for ts2 in range(2):
                    ps = pqk.tile([128, 512], f32, tag="qkps")
                    for k in range(NE):
                        nc.tensor.matmul(
                            ps[:],
                            w_sb[:, k, :],
                            xT[:, ts2 * 4 : (ts2 + 1) * 4, k, :],
                            start=(k == 0),
                            stop=(k == NE - 1),
                        )
                    # split psum->sbuf copies across ACT and DVE
                    cp = nc.scalar.copy if (wi + ts2) % 2 == tn("qksplit", 0) else (
                        lambda d, s: nc.vector.tensor_copy(d, s)
                    )
                    cp(dst[:, ts2 * 512 : (ts2 + 1) * 512], ps[:])
            return qT, kT

        qk_staged = {}

        # ---------- Phase A: load xT, compute V for all heads
        with (
            tc.tile_pool(name="wvp", bufs=1) as wvp,
            tc.tile_pool(name="pv2", bufs=tn("pv2", 3), space="PSUM") as pv2,
        ):
            wv_sb = wvp.tile([128, NE, E], bf16)
            # t-tile 0 of x first, then the first half of wv, so PE can start
            # V(ns=0) asap; remaining x and wv behind
            so = tn("sorder", 1)
            if so == 2:
                nc.sync.dma_start(xT[:, 0, 0, :], xt_d[0, :, 0:128])
                nc.sync.dma_start(wv_sb[:, 0, 0:512], wv_d[0, :, 0:512])
                nc.sync.dma_start(
                    xT[:, 0, 1:4, :].rearrange("p k n -> p (k n)"), xt_d[0, :, 128:512]
                )
                nc.sync.dma_start(
                    wv_sb[:, 1:4, 0:512],
                    wv_d[1:4, :, 0:512].rearrange("k p e -> p k e"),
                )
                nc.sync.dma_start(
                    xT[:, 0, 4:NE, :].rearrange("p k n -> p (k n)"), xt_d[0, :, 512:E]
                )
                nc.sync.dma_start(
                    wv_sb[:, 4:NE, 0:512],
                    wv_d[4:NE, :, 0:512].rearrange("k p e -> p k e"),
                )
            elif so == 0:
                nc.sync.dma_start(
                    xT[:, 0, 0:4, :].rearrange("p k n -> p (k n)"), xt_d[0, :, 0:512]
                )
                nc.sync.dma_start(
                    xT[:, 0, 4:NE, :].rearrange("p k n -> p (k n)"), xt_d[0, :, 512:E]
                )
                for k in range(NE):
                    nc.sync.dma_start(wv_sb[:, k, 0:512], wv_d[k, :, 0:512])
            else:
                nc.sync.dma_start(
                    xT[:, 0, 0:4, :].rearrange("p k n -> p (k n)"), xt_d[0, :, 0:512]
                )
                if tn("wv0first", 0):
                    nc.sync.dma_start(wv_sb[:, 0, 0:512], wv_d[0, :, 0:512])
                    nc.sync.dma_start(
                        wv_sb[:, 1:4, 0:512],
                        wv_d[1:4, :, 0:512].rearrange("k p e -> p k e"),
                    )
                else:
                    nc.sync.dma_start(
                        wv_sb[:, 0:4, 0:512],
                        wv_d[0:4, :, 0:512].rearrange("k p e -> p k e"),
                    )
                nc.sync.dma_start(
                    xT[:, 0, 4:NE, :].rearrange("p k n -> p (k n)"), xt_d[0, :, 512:E]
                )
                nc.sync.dma_start(
                    wv_sb[:, 4:NE, 0:512],
                    wv_d[4:NE, :, 0:512].rearrange("k p e -> p k e"),
                )
            for tt in range(1, NT):
                nc.sync.dma_start(
                    xT[:, tt, :, :].rearrange("p k n -> p (k n)"), xt_d[tt]
                )
            nc.sync.dma_start(
                wv_sb[:, :, 512:1024], wv_d[:, :, 512:1024].rearrange("k p e -> p k e")
            )
            nc.sync.dma_start(tri[:], tri_d[:])
            nc.sync.dma_start(id64[:], id64_d[:])
            stage_w(0)
            stage_w(1)
            nc.vector.memset(ones64[:], 1.0)
            nc.vector.memset(vaug[:, :, :, HS : HS + 1], 1.0)
            stage_lw(0)
            stage_lw(1)
            stage_lw(2)

            for ns in range(2):
                for tt in range(NT):
                    ps = pv2.tile([128, 512], f32, tag="vps")
                    for k in range(NE):
                        nc.tensor.matmul(
                            ps[:],
                            xT[:, tt, k, :],
                            wv_sb[:, k, ns * 512 : (ns + 1) * 512],
                            start=(k == 0),
                            stop=(k == NE - 1),
                        )
                    nc.vector.tensor_copy(
                        vaug[:, tt, ns * 8 : (ns + 1) * 8, 0:HS],
                        ps[:].rearrange("p (h d) -> p h d", h=8),
                    )
            qk_staged[0] = emit_qk(0)

        # ---------- Phase B: attention, one head pair at a time
        with (
            tc.tile_pool(name="ep", bufs=tn("ep", 8)) as ep,
            tc.tile_pool(name="rp", bufs=tn("rp", 4)) as rp,
            tc.tile_pool(name="sp", bufs=tn("sp", 2)) as sp,
            tc.tile_pool(name="psc", bufs=tn("psc", 3), space="PSUM") as psc,
            tc.tile_pool(name="ppv", bufs=tn("ppv", 2), space="PSUM") as ppv,
            tc.tile_pool(name="pbc", bufs=tn("pbc", 1), space="PSUM") as pbc,
        ):
            pending = []

            def emit_norm_tail(u):
                # deferred: K=1 partition-broadcast of 1/denom, then scale
                pv, rcr, pr, sub, j = u
                pb = pbc.tile([128, 512], f32, tag="pb")
                nc.tensor.matmul(
                    pb[0:64, :],
                    ones64[64:65, :],
                    rcr[64:65, :],
                    start=True,
                    stop=True,
                )
                if tn("rcbskip", 0):
                    rcb = pb
                else:
                    rcb = rp.tile([128, 512], bf16, tag="rcb")
                    nc.vector.tensor_copy(rcb[0:64, :], pb[0:64, :])
                if sub == 0:
                    nc.vector.tensor_mul(
                        outT[0:64, pr, j * 512 : (j + 1) * 512],
                        pv[0:64, :],
                        rcb[0:64, :],
                    )
                else:
                    stg = sp.tile([128, 512], bf16, tag="stg")
                    nc.vector.tensor_mul(stg[0:64, :], pv[0:64, :], rcb[0:64, :])
                    if pr == 7:
                        # last pair gates the lm head: route the partition
                        # shift through PE+DVE instead of a ~3us DMA
                        nc.tensor.matmul(
                            pb[64:128, :],
                            id64[0:64, :],
                            stg[0:64, :],
                            start=True,
                            stop=True,
                            skip_group_check=True,
                        )
                        nc.vector.tensor_copy(
                            outT[64:128, pr, j * 512 : (j + 1) * 512], pb[64:128, :]
                        )
                    else:
                        nc.sync.dma_start(
                            outT[64:128, pr, j * 512 : (j + 1) * 512], stg[0:64, :]
                        )

            for pr in range(8):
                if pr + 2 < 8:
                    stage_w(pr + 2)
                qT, kT = qk_staged.pop(pr)
                # last pair: j=1 units first so the final tail chain (which
                # gates the lm head) belongs to a cheap j=0 unit and overlaps
                # the lm runway tiles
                units = (
                    [(0, 1), (1, 1), (0, 0), (1, 0)]
                    if pr == 7
                    else [(0, 0), (0, 1), (1, 0), (1, 1)]
                )
                for sub, j in units:
                    h = 2 * pr + sub
                    q_s = qT[sub * 64 : (sub + 1) * 64, :]
                    k_s = kT[sub * 64 : (sub + 1) * 64, :]
                    if True:
                        smax = 4 * j + 3
                        e_tiles = []
                        for i in range(smax + 1):
                            ko = i - 4 * j
                            off = 128 * max(ko, 0)  # causal trim
                            ps = psc.tile([128, 512], f32, tag="sc")
                            nc.tensor.matmul(
                                ps[:, off:512],
                                k_s[:, i * 128 : (i + 1) * 128],
                                q_s[:, j * 512 + off : (j + 1) * 512],
                                start=True,
                                stop=True,
                            )
                            et = ep.tile([128, 512], bf16, tag="e")
                            nc.scalar.activation(et[:, off:512], ps[:, off:512], EXP)
                            if ko >= 0:
                                nc.gpsimd.tensor_mul(
                                    et[:, off : off + 128], et[:, off : off + 128], tri[:]
                                )
                            e_tiles.append((off, et))
                        pv = ppv.tile([128, 512], f32, tag="pv")
                        for n, (off, et) in enumerate(e_tiles):
                            nc.tensor.matmul(
                                pv[0:65, off:512],
                                vaug[:, n, h, :],
                                et[:, off:512],
                                start=(n == 0),
                                stop=(n == smax),
                                skip_group_check=True,
                            )
                        # denominator reciprocal row (stays on DVE, off PE path)
                        rc = rp.tile([128, 512], f32, tag="rc")
                        nc.vector.reciprocal(rc[64:65, :], pv[64:65, :])
                        rcr = rp.tile([128, 512], bf16, tag="rcr")
                        nc.vector.tensor_copy(rcr[64:65, :], rc[64:65, :])
                        if pending and (pending[0][2] < 7 or len(pending) >= 2):
                            emit_norm_tail(pending.pop(0))
                        pending.append((pv, rcr, pr, sub, j))
                if pr + 1 < 8:
                    qk_staged[pr + 1] = emit_qk(pr + 1)

            def lm_mms(ps, v, tt):
                lw = lw_staged[v]
                for k in range(NE):
                    nc.tensor.matmul(
                        ps[:, 0:NVC],
                        outT[:, k, tt * 128 : (tt + 1) * 128],
                        lw[:, k, :],
                        start=(k == 0),
                        stop=(k == NE - 1),
                    )

            def lm_og(ps, v, tt, last=False):
                bias = bias_staged[v]
                og = ogp.tile([128, NVC], bf16, tag="og")
                dst = out_d[tt * 128 : (tt + 1) * 128, v * NVC : (v + 1) * NVC]
                if not last:
                    nc.vector.tensor_add(og[:], ps[:, 0:NVC], bias[:])
                    nc.sync.dma_start(dst, og[:])
                else:
                    # split the final tile so its out-DMA overlaps the add
                    nc.vector.tensor_add(og[:, 0:250], ps[:, 0:250], bias[:, 0:250])
                    nc.sync.dma_start(dst[:, 0:250], og[:, 0:250])
                    nc.vector.tensor_add(og[:, 250:NVC], ps[:, 250:NVC], bias[:, 250:NVC])
                    nc.sync.dma_start(dst[:, 250:NVC], og[:, 250:NVC])

            def lm_tile_ps(ps, v, tt, last=False):
                lm_mms(ps, v, tt)
                lm_og(ps, v, tt, last)

            # lm runway inside phase B: t-tiles 4:8 of v0/v1 depend only on
            # the j=1 attention outputs, which the (reordered) last pair
            # finishes first. They reuse the now-idle qk psum ring so the
            # final (j=0) norm chain overlaps lm work instead of stalling PE.
            # The og adds trail the psum ring by 2 so the final norm tail's
            # DVE chain (which gates phase C's t-tiles 0:4) jumps the queue.
            rw = []
            for v in range(tn("rwv", 2)):
                for tt in range(4, NT):
                    ps = pqk.tile([128, 512], f32, tag="qkps")
                    lm_mms(ps, v, tt)
                    rw.append((ps, v, tt))
                    if len(rw) in (2, 4) and pending:
                        emit_norm_tail(pending.pop(0))
                    if len(rw) > 2:
                        lm_og(*rw[len(rw) - 3])
            while pending:
                emit_norm_tail(pending.pop(0))
            for args in rw[-2:]:
                lm_og(*args)

        # ---------- Phase C: lm head
        with tc.tile_pool(name="plm", bufs=tn("plm", 6), space="PSUM") as plm:
            def lm_tile(v, tt, last=False):
                ps = plm.tile([128, NVC], f32, tag="lm")
                lm_tile_ps(ps, v, tt, last)

            for v in range(tn("rwv", 2), 2):
                for tt in range(4, NT):
                    lm_tile(v, tt)
            for v in range(NVT):
                if v + 3 < NVT:
                    stage_lw(v + 3)
                for tt in range(0, 4 if v < 2 else NT):
                    lm_tile(v, tt, last=(v == NVT - 1 and tt == (4 if v < 2 else NT) - 1))
                lw_staged.pop(v)
                bias_staged.pop(v)

    nc.compile()
    return nc


def _prep_shared(tok_emb, pos_emb, Wq, Wk, Wv):
    import ml_dtypes

    bf = ml_dtypes.bfloat16

    def pair_stack(W):
        out = np.empty((8, 128, NE, 128), dtype=bf)
        for p in range(8):
            pairw = np.concatenate([W[2 * p], W[2 * p + 1]], axis=1)  # [E, 128]
            out[p] = pairw.reshape(NE, 128, 128).transpose(1, 0, 2).astype(bf)
        return out

    wq = pair_stack(np.asarray(Wq, np.float32))
    wk = pair_stack(np.asarray(Wk, np.float32))
    wv = np.ascontiguousarray(
        np.asarray(Wv, np.float32)
        .transpose(1, 0, 2)
        .reshape(E, H * HS)
        .reshape(NE, 128, E)
        .astype(bf)
    )
    p_idx = np.arange(128)[:, None]
    c_idx = np.arange(128)[None, :]
    tri = (p_idx <= c_idx).astype(bf)
    id64 = np.zeros((128, 64), dtype=bf)
    id64[np.arange(64), np.arange(64)] = 1.0
    return {"wq": wq, "wk": wk, "wv": wv, "tri": tri, "id64": id64}


def build_in_maps(inputs):
    import ml_dtypes

    bf = ml_dtypes.bfloat16
    idx = np.asarray(inputs["idx"])
    tok = np.asarray(inputs["tok_emb"], dtype=np.float32)
    pos = np.asarray(inputs["pos_emb"], dtype=np.float32)
    lm_W = np.asarray(inputs["lm_W"], dtype=np.float32)
    lm_b = np.asarray(inputs["lm_b"], dtype=np.float32)
    shared = _prep_shared(
        inputs["tok_emb"], inputs["pos_emb"], inputs["Wq"], inputs["Wk"], inputs["Wv"]
    )
    in_maps = []
    for c in range(8):
        b, v = c // 2, c % 2
        m = dict(shared)
        # host-side embedding: xT[tt, e_row, (k|n)] = x[b, tt*128+n, ...]
        x = tok[idx[b]] + pos  # [T, E]
        xt = np.ascontiguousarray(
            x.T.reshape(NE, 128, NT, 128).transpose(2, 1, 0, 3).reshape(NT, 128, E)
        ).astype(bf)
        m["xt"] = xt
        m["lmw"] = np.ascontiguousarray(lm_W[:, v * VSH : (v + 1) * VSH].astype(bf))
        m["lmb"] = np.ascontiguousarray(lm_b[v * VSH : (v + 1) * VSH].reshape(1, VSH))
        in_maps.append(m)
    return in_maps


def kernel(idx, tok_emb, pos_emb, Wq, Wk, Wv, lm_W, lm_b):
    from concourse.bass_utils import run_bass_kernel_spmd

    if "nc" not in _cache:
        _cache["nc"] = _build_nc()
    nc = _cache["nc"]

    in_maps = build_in_maps(
        dict(
            idx=idx,
            tok_emb=tok_emb,
            pos_emb=pos_emb,
            Wq=Wq,
            Wk=Wk,
            Wv=Wv,
            lm_W=lm_W,
            lm_b=lm_b,
        )
    )

    res = run_bass_kernel_spmd(nc, in_maps, core_ids=list(range(8)))
    logits = np.empty((B, T, VOCAB), np.float32)
    for c in range(8):
        b, v = c // 2, c % 2
        logits[b, :, v * VSH : (v + 1) * VSH] = res.results[c]["logits"].astype(
            np.float32
        )
    return logits
